# revision 7
# baseline (speedup 1.0000x reference)
"""Trainium2 Bass kernel for nn_DetectionLoss (YOLO-style detection loss).

Strategy (pure data parallel over 8 NeuronCores, 256 images each):
  - Dense-load det in [128ch, img*169cell] chunks (memory roofline).
  - One GPSIMD ap_gather per chunk pulls all 128 channel-partitions at the
    32 object cells per image -> G[128ch, obj].
  - PE transposes G into object-major GT[128obj, ch] tiles (ACT copies).
  - DVE does IoU / argmax / last-writer-wins dedup / loss assembly,
    pipelined in 4 passes so math overlaps the DMA stream.
  - Dense no-obj conf sum via a separate per-anchor reload of channel 4.
  - Output: per-core partial sums [128, 16]; host reduces across cores.
"""
import numpy as np

GRID = 13
NA = 5
NCLS = 20
CH = 25
NCH = NA * CH          # 125
CELLS = GRID * GRID    # 169
O = 32                 # objects per image
B = 2048               # global batch
NCORES = 8
BLOC = B // NCORES     # 256 images per core
C_IMG = 32             # images per chunk
NCHUNK = BLOC // C_IMG                   # 8
NE = C_IMG * CELLS                       # 5408 elems/partition per chunk
NIDX = C_IMG * O                         # 1024 gather idxs per chunk
NOBJ = BLOC * O                          # 8192 objects per core
J2 = NOBJ // 128                         # 64 object columns
NPASS = 4
CPP = NCHUNK // NPASS                    # chunks per pass (2)
JPP = J2 // NPASS                        # j2 per pass (16)

ANCHORS = np.array([1.3221, 1.73145, 3.19275, 4.00944, 5.05587,
                    8.09892, 9.47112, 4.84053, 11.2364, 10.0071],
                   dtype=np.float32)

_CACHE = {}


def _make_consts():
    """Host-precomputed, data-independent constant input tensors."""
    consts = {}
    consts["c_ident"] = np.eye(128, dtype=np.float32)
    # 8 partition-selector matrices for the idx shuffle, packed [128, 8*128].
    # matmul r: out_r[p, :] = k_obj[16r + p%16, :]
    sel = np.zeros((128, 8 * 128), dtype=np.float32)
    for r in range(8):
        for p in range(128):
            sel[16 * r + (p % 16), r * 128 + p] = 1.0
    consts["c_sel"] = sel
    consts["c_iota5"] = np.tile(np.arange(5, dtype=np.float32), (128, 1))
    consts["c_iota5m"] = np.tile(np.arange(5, dtype=np.float32) - 99.0, (128, 1))
    consts["c_iota20"] = np.tile(np.arange(NCLS, dtype=np.float32), (128, 1))
    consts["c_s2"] = np.tile((ANCHORS[0::2] / GRID).astype(np.float32), (128, 1))
    consts["c_s3"] = np.tile((ANCHORS[1::2] / GRID).astype(np.float32), (128, 1))
    # strict upper-triangular pair mask over (o, o2): 1.0 iff o2 > o
    tri = (np.arange(O)[None, :] > np.arange(O)[:, None]).astype(np.float32)
    consts["c_tri"] = np.tile(tri.reshape(1, O * O), (128, 1))
    # imgbase[p, c*64+s] = 169 * (s // 2)  (img_local of wrapped idx slot)
    ib = np.zeros((128, NCHUNK * 64), dtype=np.float32)
    for s in range(64):
        ib[:, np.arange(NCHUNK) * 64 + s] = float(CELLS * (s // 2))
    consts["c_imgbase"] = ib
    return consts


def _build(repeat=1, stage=5, sub=9):
    """Build the Bass module (emitted once, cached)."""
    import concourse.bacc as bacc
    import concourse.tile as tile
    from concourse import mybir

    f32 = mybir.dt.float32
    i16 = mybir.dt.int16
    ALU = mybir.AluOpType
    AX = mybir.AxisListType
    ACT = mybir.ActivationFunctionType

    from concourse.ap import AP

    nc = bacc.Bacc(None, target_bir_lowering=False, debug=False)

    # one pad image at the end makes the 128-partition wrapped chunk AP
    # (partition stride 169 over 128 > 125 channels) stay in bounds
    det = nc.dram_tensor("det", [BLOC + 1, NCH, CELLS], f32, kind="ExternalInput")
    gtb = nc.dram_tensor("gtb", [BLOC, O, 4], f32, kind="ExternalInput")
    clsf = nc.dram_tensor("clsf", [BLOC, O], f32, kind="ExternalInput")
    c_ident = nc.dram_tensor("c_ident", [128, 128], f32, kind="ExternalInput")
    c_sel = nc.dram_tensor("c_sel", [128, 8 * 128], f32, kind="ExternalInput")
    c_iota5 = nc.dram_tensor("c_iota5", [128, 5], f32, kind="ExternalInput")
    c_iota5m = nc.dram_tensor("c_iota5m", [128, 5], f32, kind="ExternalInput")
    c_iota20 = nc.dram_tensor("c_iota20", [128, NCLS], f32, kind="ExternalInput")
    c_s2 = nc.dram_tensor("c_s2", [128, 5], f32, kind="ExternalInput")
    c_s3 = nc.dram_tensor("c_s3", [128, 5], f32, kind="ExternalInput")
    c_tri = nc.dram_tensor("c_tri", [128, O * O], f32, kind="ExternalInput")
    c_imgbase = nc.dram_tensor("c_imgbase", [128, NCHUNK * 64], f32,
                               kind="ExternalInput")
    out = nc.dram_tensor("out", [128, 16], f32, kind="ExternalOutput")

    with tile.TileContext(nc) as tc:
        with tc.tile_pool(name="cpool", bufs=1) as cp, \
             tc.tile_pool(name="work", bufs=1) as wk, \
             tc.tile_pool(name="psA", bufs=2, space="PSUM") as psA, \
             tc.tile_pool(name="psB", bufs=3, space="PSUM") as psB:

            # ---- constants into SBUF ----
            t_id = cp.tile([128, 128], f32)
            t_sel = cp.tile([128, 8 * 128], f32)
            t_i5 = cp.tile([128, 5], f32)
            t_i5m = cp.tile([128, 5], f32)
            t_i20 = cp.tile([128, NCLS], f32)
            t_s2c = cp.tile([128, 5], f32)
            t_s3c = cp.tile([128, 5], f32)
            t_tri = cp.tile([128, O * O], f32)
            t_ib = cp.tile([128, NCHUNK * 64], f32)
            nc.sync.dma_start(t_id[:], c_ident[:])
            nc.sync.dma_start(t_sel[:], c_sel[:])
            nc.sync.dma_start(t_i5[:], c_iota5[:])
            nc.sync.dma_start(t_i5m[:], c_iota5m[:])
            nc.sync.dma_start(t_i20[:], c_iota20[:])
            nc.sync.dma_start(t_s2c[:], c_s2[:])
            nc.sync.dma_start(t_s3c[:], c_s3[:])
            nc.sync.dma_start(t_tri[:], c_tri[:])
            nc.sync.dma_start(t_ib[:], c_imgbase[:])

            # ---- gt loads, object-major: object n = b*32+o = j2*128 + p ----
            # p = (b%4)*32 + o, j2 = b//4
            t_gtb = wk.tile([128, J2 * 4], f32)
            if sub >= 2: nc.sync.dma_start(
                t_gtb[:].rearrange("p (j c) -> p j c", c=4),
                gtb[:].rearrange("(j bi) o c -> (bi o) j c", bi=4))
            t_cls = wk.tile([128, J2], f32)
            if sub >= 2: nc.sync.dma_start(
                t_cls[:], clsf[:].rearrange("(j bi) o -> (bi o) j", bi=4))

            gv = t_gtb[:].rearrange("p (j c) -> p j c", c=4)
            x_ap = gv[:, :, 0]
            y_ap = gv[:, :, 1]
            w_ap = gv[:, :, 2]
            h_ap = gv[:, :, 3]

            # ---- cell coords (DVE, object-major [128, 64]) ----
            t_mx = wk.tile([128, J2], f32)
            t_my = wk.tile([128, J2], f32)
            t_tx = wk.tile([128, J2], f32)
            t_ty = wk.tile([128, J2], f32)
            t_gx = wk.tile([128, J2], f32)
            t_gy = wk.tile([128, J2], f32)
            t_k = wk.tile([128, J2], f32)
            t_scr0 = wk.tile([128, J2], f32)
            if sub >= 3:
              nc.vector.tensor_scalar_mul(t_mx[:], x_ap, float(GRID))
              nc.vector.tensor_scalar_mul(t_my[:], y_ap, float(GRID))
            # floor(v), robust to the fp->int rounding mode:
            #   i = cvt(v); fi = cvt_back(i); gx = fi - (fi > v)
            t_i32 = wk.tile([128, J2], mybir.dt.int32)
            for t_m_, t_g_ in (((t_mx, t_gx), (t_my, t_gy)) if sub >= 3 else ()):
                nc.vector.tensor_copy(t_i32[:], t_m_[:])
                nc.vector.tensor_copy(t_g_[:], t_i32[:])
                nc.vector.tensor_tensor(t_scr0[:], t_g_[:], t_m_[:], ALU.is_gt)
                nc.vector.tensor_sub(t_g_[:], t_g_[:], t_scr0[:])
            if sub >= 3:
              nc.vector.tensor_sub(t_tx[:], t_mx[:], t_gx[:])
              nc.vector.tensor_sub(t_ty[:], t_my[:], t_gy[:])
              nc.vector.scalar_tensor_tensor(
                  out=t_k[:], in0=t_gy[:], scalar=float(GRID), in1=t_gx[:],
                  op0=ALU.mult, op1=ALU.add)

            # ---- gather-index shuffle into ap_gather's wrapped layout ----
            # idx16[p, c*64+s] = img(s)*169 + k[obj n = c*1024 + 16s + p%16]
            # source (q = 16*(s%8) + p%16, j2 = c*8 + s//8)
            t_idxf = wk.tile([128, NCHUNK * 64], f32)
            for r in range(8 if sub >= 4 else 0):
                t_pr = psA.tile([128, J2], f32, space="PSUM", tag="shuf")
                nc.tensor.matmul(
                    out=t_pr[:], lhsT=t_sel[:, r * 128:(r + 1) * 128],
                    rhs=t_k[:], start=True, stop=True)
                nc.scalar.activation(
                    t_idxf[:].rearrange("p (c sd r) -> p c sd r", sd=8, r=8)
                    [:, :, :, r],
                    t_pr[:].rearrange("p (c sd) -> p c sd", sd=8),
                    ACT.Copy)
            t_idx16 = wk.tile([128, NCHUNK * 64], i16)
            if sub >= 5:
                nc.vector.tensor_add(t_idxf[:], t_idxf[:], t_ib[:])
                nc.vector.tensor_copy(t_idx16[:], t_idxf[:])

            # ---- persistent big tiles ----
            t_T0 = wk.tile([128, NE], f32)
            t_T1 = wk.tile([128, NE], f32)
            t_G = wk.tile([128, NOBJ], f32)
            t_GT = wk.tile([128, J2 * NCH], f32)
            t_GSQ = wk.tile([128, J2 * NCH], f32)


            t_stage = wk.tile([128, 16], f32)
            t_red1 = wk.tile([128, 1], f32)
            nc.vector.memset(t_stage[:], 0.0)

            # per-pass work tiles (reused across passes)
            def w5():
                return wk.tile([128, JPP * NA], f32,
                               name=f"w5_{nc.next_id()}")
            t_iou = w5(); t_scr = w5(); t_scr2 = w5()
            t_pw = w5(); t_ph = w5()
            t_bx0 = w5(); t_by0 = w5(); t_bx1 = w5(); t_by1 = w5()
            t_ix0 = w5(); t_iy0 = w5(); t_inter = w5(); t_den = w5()
            t_ohA = w5(); t_W = w5(); t_qcls = w5(); t_s2s = w5()
            t_csse = w5(); t_c1 = w5()

            def w1(nm):
                return wk.tile([128, JPP], f32, name=nm)
            t_hw2 = w1("t_hw2"); t_hh2 = w1("t_hh2")
            t_gx0 = w1("t_gx0"); t_gy0 = w1("t_gy0")
            t_gx1 = w1("t_gx1"); t_gy1 = w1("t_gy1")
            t_a1 = w1("t_a1"); t_mm = w1("t_mm")
            t_aidx = w1("t_aidx"); t_sid = w1("t_sid"); t_win = w1("t_win")
            t_sT = wk.tile([JPP, 128], f32)
            t_eqp = wk.tile([JPP, O * O * 4], f32)
            t_dead = wk.tile([JPP, 128], f32)
            t_oh = wk.tile([128, JPP * NCLS], f32)
            t_qc = wk.tile([128, JPP * NA * NCLS], f32)
            t_diff = wk.tile([128, JPP * NA * 4], f32)

            def r5(t):
                return t[:].rearrange("p (j a) -> p j a", a=NA)

            import contextlib
            rep_ctx = tc.For_i(0, repeat, 1) if repeat > 1 \
                else contextlib.nullcontext()
            with rep_ctx:
              for ps in range(NPASS if stage >= 2 else 0):
                  # ---- load + gather + transpose the pass's chunks ----
                  for cc in range(CPP):
                      c = ps * CPP + cc
                      t_T = t_T0 if (c % 2 == 0) else t_T1
                      # 128-partition wrapped src AP: partition p = channel p
                      # (p>=125 wraps into the next image's ch 0-2, ignored
                      # downstream). 128 partitions -> 16 DMA rings vs 5.
                      src = AP(det, c * C_IMG * NCH * CELLS,
                               [[CELLS, 128], [NCH * CELLS, C_IMG], [1, CELLS]])
                      nc.sync.dma_start(
                          t_T[:].rearrange("p (i e) -> p i e", e=CELLS), src)
                      nc.gpsimd.ap_gather(
                          out_ap=t_G[:, c * NIDX:(c + 1) * NIDX],
                          in_ap=t_T[:],
                          idxs_ap=t_idx16[:, c * 64:(c + 1) * 64],
                          channels=128, num_elems=NE, d=1, num_idxs=NIDX)
                      for j in range(NIDX // 128 if stage >= 3 else 0):
                          base = c * NIDX + j * 128
                          t_tp = psB.tile([128, 128], f32, space="PSUM", tag="tp")
                          nc.tensor.transpose(
                              out=t_tp[:], in_=t_G[:, base:base + 128],
                              identity=t_id[:])
                          jg = c * 8 + j
                          nc.scalar.activation(
                              t_GT[:, jg * NCH:(jg + 1) * NCH],
                              t_tp[:, 0:NCH], ACT.Copy)
                  lo, hi = ps * JPP * NCH, (ps + 1) * JPP * NCH
                  if stage < 3:
                      continue
                  nc.scalar.activation(t_GSQ[:, lo:hi], t_GT[:, lo:hi], ACT.Square)

                  if stage < 4:
                      continue
                  jsl = slice(ps * JPP, (ps + 1) * JPP)
                  gtv = t_GT[:, lo:hi].rearrange("p (j a r) -> p j a r", a=NA, r=CH)
                  gsv = t_GSQ[:, lo:hi].rearrange("p (j a r) -> p j a r", a=NA, r=CH)
                  q0 = gtv[:, :, :, 0]
                  q1 = gtv[:, :, :, 1]
                  q2 = gtv[:, :, :, 2]
                  q3 = gtv[:, :, :, 3]
                  q4 = gtv[:, :, :, 4]
                  q4sq = gsv[:, :, :, 4]
                  qclsv = gtv[:, :, :, 5:CH]          # [p, j, a, 20]
                  qclssq = gsv[:, :, :, 5:CH]

                  def b5(ap2d):  # [128, JPP] -> broadcast [128, JPP, 5]
                      return ap2d.rearrange("p (j one) -> p j one", one=1) \
                                 .to_broadcast([128, JPP, NA])

                  def c5(tile1):  # const [128, 5] -> [128, JPP, 5]
                      return tile1[:].rearrange("p (one a) -> p one a", one=1) \
                                     .to_broadcast([128, JPP, NA])

                  # ---- IoU (per object x anchor) ----
                  nc.vector.tensor_tensor(r5(t_pw), q2, c5(t_s2c), ALU.mult)
                  nc.vector.tensor_tensor(r5(t_ph), q3, c5(t_s3c), ALU.mult)
                  # bx0 = (px+gx)/13 - pw/2 ; by0 = (py+gy)/13 - ph/2
                  nc.vector.tensor_tensor(r5(t_bx0), q0, b5(t_gx[:, jsl]), ALU.add)
                  nc.vector.tensor_scalar_mul(t_bx0[:], t_bx0[:], 1.0 / GRID)
                  nc.vector.scalar_tensor_tensor(
                      out=t_bx0[:], in0=t_pw[:], scalar=-0.5, in1=t_bx0[:],
                      op0=ALU.mult, op1=ALU.add)
                  nc.vector.tensor_tensor(r5(t_by0), q1, b5(t_gy[:, jsl]), ALU.add)
                  nc.vector.tensor_scalar_mul(t_by0[:], t_by0[:], 1.0 / GRID)
                  nc.vector.scalar_tensor_tensor(
                      out=t_by0[:], in0=t_ph[:], scalar=-0.5, in1=t_by0[:],
                      op0=ALU.mult, op1=ALU.add)
                  nc.vector.tensor_add(t_bx1[:], t_bx0[:], t_pw[:])
                  nc.vector.tensor_add(t_by1[:], t_by0[:], t_ph[:])
                  # gt box corners [128, JPP]
                  nc.vector.tensor_scalar_mul(t_hw2[:], w_ap[:, jsl], 0.5)
                  nc.vector.tensor_scalar_mul(t_hh2[:], h_ap[:, jsl], 0.5)
                  nc.vector.tensor_sub(t_gx0[:], x_ap[:, jsl], t_hw2[:])
                  nc.vector.tensor_add(t_gx1[:], x_ap[:, jsl], t_hw2[:])
                  nc.vector.tensor_sub(t_gy0[:], y_ap[:, jsl], t_hh2[:])
                  nc.vector.tensor_add(t_gy1[:], y_ap[:, jsl], t_hh2[:])
                  # a1 = (gx1-gx0+1)*(gy1-gy0+1)
                  nc.vector.tensor_sub(t_a1[:], t_gx1[:], t_gx0[:])
                  nc.vector.tensor_scalar_add(t_a1[:], t_a1[:], 1.0)
                  nc.vector.tensor_sub(t_mm[:], t_gy1[:], t_gy0[:])
                  nc.vector.tensor_scalar_add(t_mm[:], t_mm[:], 1.0)
                  nc.vector.tensor_mul(t_a1[:], t_a1[:], t_mm[:])
                  # intersection: ix0 = max(gx0,bx0); ix1 = min(gx1,bx1) (in bx1)
                  nc.vector.tensor_tensor(r5(t_ix0), r5(t_bx0), b5(t_gx0), ALU.max)
                  nc.vector.tensor_tensor(r5(t_iy0), r5(t_by0), b5(t_gy0), ALU.max)
                  nc.vector.tensor_tensor(r5(t_bx1), r5(t_bx1), b5(t_gx1), ALU.min)
                  nc.vector.tensor_tensor(r5(t_by1), r5(t_by1), b5(t_gy1), ALU.min)
                  nc.vector.tensor_sub(t_bx1[:], t_bx1[:], t_ix0[:])
                  nc.vector.tensor_scalar_add(t_bx1[:], t_bx1[:], 1.0)
                  nc.vector.tensor_sub(t_by1[:], t_by1[:], t_iy0[:])
                  nc.vector.tensor_scalar_add(t_by1[:], t_by1[:], 1.0)
                  nc.vector.tensor_mul(t_inter[:], t_bx1[:], t_by1[:])
                  # a2 = (pw+1)*(ph+1); denom = a1 + a2 - inter
                  nc.vector.tensor_scalar_add(t_pw[:], t_pw[:], 1.0)
                  nc.vector.tensor_scalar_add(t_ph[:], t_ph[:], 1.0)
                  nc.vector.tensor_mul(t_den[:], t_pw[:], t_ph[:])
                  nc.vector.tensor_tensor(r5(t_den), r5(t_den), b5(t_a1), ALU.add)
                  nc.vector.tensor_sub(t_den[:], t_den[:], t_inter[:])
                  nc.vector.reciprocal(t_den[:], t_den[:])
                  nc.vector.tensor_mul(t_iou[:], t_inter[:], t_den[:])

                  # ---- argmax over anchors (first max wins) ----
                  nc.vector.reduce_max(t_mm[:], r5(t_iou), axis=AX.X)
                  nc.vector.tensor_tensor(
                      r5(t_scr), r5(t_iou), b5(t_mm), ALU.is_equal)
                  nc.vector.tensor_tensor(
                      r5(t_scr2), r5(t_scr), c5(t_i5m), ALU.mult)
                  nc.vector.tensor_reduce(
                      t_aidx[:], r5(t_scr2), axis=AX.X, op=ALU.min)
                  nc.vector.tensor_scalar_add(t_aidx[:], t_aidx[:], 99.0)

                  # ---- slot id s = 169*aidx + k ; last-writer-wins dedup ----
                  nc.vector.scalar_tensor_tensor(
                      out=t_sid[:], in0=t_aidx[:], scalar=float(CELLS),
                      in1=t_k[:, jsl], op0=ALU.mult, op1=ALU.add)
                  t_tp1 = psA.tile([JPP, 128], f32, space="PSUM", tag="ded", bufs=1)
                  nc.tensor.transpose(out=t_tp1[:], in_=t_sid[:], identity=t_id[:])
                  nc.scalar.activation(t_sT[:], t_tp1[:], ACT.Copy)
                  sTa = t_sT[:].rearrange("p (i o one) -> p i o one", i=4, one=1) \
                               .to_broadcast([JPP, 4, O, O])
                  sTb = t_sT[:].rearrange("p (i one o2) -> p i one o2", i=4, one=1) \
                               .to_broadcast([JPP, 4, O, O])
                  eqv = t_eqp[:].rearrange("p (i o o2) -> p i o o2", i=4, o2=O)
                  nc.vector.tensor_tensor(eqv, sTa, sTb, ALU.is_equal)
                  triv = t_tri[0:JPP, :].rearrange(
                      "p (one o o2) -> p one o o2", one=1, o2=O) \
                      .to_broadcast([JPP, 4, O, O])
                  nc.vector.tensor_tensor(eqv, eqv, triv, ALU.mult)
                  nc.vector.tensor_reduce(
                      t_dead[:].rearrange("p (i o) -> p i o", o=O),
                      eqv, axis=AX.X, op=ALU.max)
                  t_tp2 = psA.tile([128, JPP], f32, space="PSUM", tag="ded2", bufs=1)
                  nc.tensor.transpose(
                      out=t_tp2[:], in_=t_dead[:], identity=t_id[0:JPP, 0:JPP])
                  nc.scalar.activation(t_win[:], t_tp2[:], ACT.Copy)
                  nc.vector.tensor_scalar(
                      t_win[:], t_win[:], -1.0, 1.0, ALU.mult, ALU.add)

                  # ---- masks: W = onehot(aidx) * win ----
                  nc.vector.tensor_tensor(
                      r5(t_ohA), b5(t_aidx), c5(t_i5), ALU.is_equal)
                  nc.vector.tensor_tensor(
                      r5(t_W), r5(t_ohA), b5(t_win), ALU.mult)

                  # ---- coord SSE ----
                  dv = t_diff[:].rearrange("p (j a c) -> p j a c", a=NA, c=4)
                  nc.vector.tensor_tensor(
                      dv[:, :, :, 0], q0, b5(t_tx[:, jsl]), ALU.subtract)
                  nc.vector.tensor_tensor(
                      dv[:, :, :, 1], q1, b5(t_ty[:, jsl]), ALU.subtract)
                  nc.vector.tensor_tensor(r5(t_scr), q2, c5(t_s2c), ALU.mult)
                  nc.vector.tensor_tensor(
                      dv[:, :, :, 2], r5(t_scr), b5(w_ap[:, jsl]), ALU.subtract)
                  nc.vector.tensor_tensor(r5(t_scr), q3, c5(t_s3c), ALU.mult)
                  nc.vector.tensor_tensor(
                      dv[:, :, :, 3], r5(t_scr), b5(h_ap[:, jsl]), ALU.subtract)
                  nc.vector.tensor_mul(t_diff[:], t_diff[:], t_diff[:])
                  nc.vector.tensor_reduce(r5(t_csse), dv, axis=AX.X, op=ALU.add)

                  # ---- conf terms: (1-q4)^2 ----
                  nc.vector.tensor_scalar(
                      r5(t_c1), q4, -1.0, 1.0, ALU.mult, ALU.add)
                  nc.vector.tensor_mul(t_c1[:], t_c1[:], t_c1[:])

                  # ---- class terms ----
                  ohv = t_oh[:].rearrange("p (j c) -> p j c", c=NCLS)
                  nc.vector.tensor_tensor(
                      ohv,
                      t_cls[:, jsl].rearrange("p (j one) -> p j one", one=1)
                      .to_broadcast([128, JPP, NCLS]),
                      t_i20[:].rearrange("p (one c) -> p one c", one=1)
                      .to_broadcast([128, JPP, NCLS]),
                      ALU.is_equal)
                  qcv = t_qc[:].rearrange("p (j a c) -> p j a c", a=NA, c=NCLS)
                  for a in range(NA):
                      nc.vector.tensor_tensor(
                          qcv[:, :, a, :], qclsv[:, :, a, :], ohv, ALU.mult)
                  nc.vector.tensor_reduce(
                      r5(t_qcls), qcv, axis=AX.X, op=ALU.add)
                  nc.vector.tensor_reduce(
                      r5(t_s2s), qclssq, axis=AX.X, op=ALU.add)
                  # cls_t = S2 - 2*qcls  (the +1 handled via sum(W))
                  nc.vector.scalar_tensor_tensor(
                      out=t_s2s[:], in0=t_qcls[:], scalar=-2.0, in1=t_s2s[:],
                      op0=ALU.mult, op1=ALU.add)

                  # ---- accumulate masked partial sums into staging ----
                  def acc(col, in0_ap):
                      nc.vector.tensor_tensor(
                          r5(t_scr2), in0_ap, r5(t_W), ALU.mult)
                      nc.vector.reduce_sum(
                          t_red1[:], t_scr2[:], axis=AX.X)
                      nc.vector.tensor_add(
                          t_stage[:, col:col + 1], t_stage[:, col:col + 1],
                          t_red1[:])
                  acc(0, r5(t_csse))    # coord SSE (unweighted by 5)
                  acc(1, r5(t_c1))      # (1-q4)^2 at slots
                  acc(2, q4sq)          # q4^2 at slots
                  acc(3, r5(t_s2s))     # S2 - 2*qcls at slots
                  acc(4, r5(t_W))       # sum(W) (W*W = W)

              # ---- dense conf sum: reload ch4 per anchor, square+reduce ----
              t_cfs = wk.tile([128, 2 * CELLS], f32)
              if stage == 1:
                  if sub >= 4:
                      nc.vector.tensor_copy(t_stage[:], t_idxf[:, 0:16])
                  elif sub >= 2:
                      nc.vector.tensor_copy(t_stage[:], t_gtb[:, 0:16])
                  else:
                      nc.vector.tensor_copy(t_stage[:], t_id[:, 0:16])
              elif stage == 2:
                  nc.vector.tensor_copy(t_stage[:], t_G[:, 0:16])
              elif stage == 3:
                  nc.vector.tensor_copy(t_stage[:], t_GT[:, 0:16])
              for a in range(NA if stage >= 5 else 0):
                  t_cf = wk.tile([128, 2 * CELLS], f32, tag="cf", bufs=2,
                                 name=f"t_cf{a}")
                  nc.sync.dma_start(
                      t_cf[:].rearrange("p (bh e) -> p bh e", e=CELLS),
                      det[0:BLOC, a * CH + 4, :]
                      .rearrange("(bh p) e -> p bh e", p=128))
                  nc.vector.tensor_mul(t_cfs[:], t_cf[:], t_cf[:])
                  nc.vector.reduce_sum(
                      t_stage[:, 5 + a:6 + a], t_cfs[:], axis=AX.X)

            nc.sync.dma_start(out[:], t_stage[:])

    nc.compile()
    return nc


def _get_built():
    if "nc" not in _CACHE:
        _CACHE["nc"] = _build()
        _CACHE["consts"] = _make_consts()
    return _CACHE["nc"], _CACHE["consts"]


def _reduce_partials(P):
    """P: [ncores, 128, 16] fp32 partials -> the 4 scalar losses."""
    S = P.astype(np.float64).sum(axis=(0, 1))
    coord, confobj, confsub, clsq, wsum = S[0], S[1], S[2], S[3], S[4]
    dense = S[5:10].sum()
    obj_loss = 5.0 * coord + confobj
    no_obj_loss = 0.5 * (dense - confsub)
    conf_loss = clsq + wsum
    loss = obj_loss + no_obj_loss + conf_loss
    return (np.float32(loss), np.float32(obj_loss),
            np.float32(no_obj_loss), np.float32(conf_loss))


def kernel(detection_result, gt_boxes, gt_class):
    from concourse.bass_utils import run_bass_kernel_spmd

    nc, consts = _get_built()
    det = np.ascontiguousarray(
        np.asarray(detection_result, dtype=np.float32)).reshape(B, NCH, CELLS)
    gtb = np.ascontiguousarray(np.asarray(gt_boxes, dtype=np.float32))
    clsf = np.asarray(gt_class).astype(np.float32)

    in_maps = []
    for c in range(NCORES):
        sl = slice(c * BLOC, (c + 1) * BLOC + 1)
        if c < NCORES - 1:
            dslice = det[sl]  # contiguous view incl. one pad image
        else:
            dslice = np.concatenate(
                [det[c * BLOC:], det[-1:]], axis=0)  # pad last core
        m = {"det": dslice, "gtb": gtb[c * BLOC:(c + 1) * BLOC],
             "clsf": clsf[c * BLOC:(c + 1) * BLOC]}
        m.update(consts)
        in_maps.append(m)

    res = run_bass_kernel_spmd(nc, in_maps, core_ids=list(range(NCORES)))
    _CACHE["last_res"] = res
    P = np.stack([res.results[c]["out"] for c in range(NCORES)])
    return _reduce_partials(P)



# revision 14
# speedup vs baseline: 1.7552x; 1.7552x over previous
"""Trainium2 Bass kernel for nn_DetectionLoss (YOLO-style detection loss).

Strategy (pure data parallel over 8 NeuronCores, 256 images each):
  - Host reformats det [B,125,13,13] into cell-major detT rows [169*B, 128]
    (zero-padded ch) plus a compact conf slice [B,5,169].
  - The loss reads only ~23% of det: per core, 64 hardware indirect DMAs
    (qPoolDynamic, one 512B row per partition per call) pull the 8192
    object rows straight from HBM into an object-major [128, 64*128] tile;
    after the anchor argmax a second round of 64 indirect DMAs fetches the
    winner's 32-element sub-row for the loss terms. One dense 865KB load
    covers the no-object conf term. ~5.2MB HBM/core vs 22.5MB dense.
  - DVE does IoU / argmax / winner terms; one single-shot pairwise
    last-writer-wins dedup on [64, 4096] overlaps the round-2 gathers.
  - Output: per-core partial sums [128, 16]; host reduces across cores.
"""
import numpy as np

GRID = 13
NA = 5
NCLS = 20
CH = 25
NCH = NA * CH          # 125
CHP = 128              # padded channel dim in detT rows
CELLS = GRID * GRID    # 169
O = 32                 # objects per image
B = 2048               # global batch
NCORES = 8
BLOC = B // NCORES     # 256 images per core
NOBJ = BLOC * O        # 8192 objects per core
J2 = NOBJ // 128       # 64 object columns
WIN = 32               # winner sub-row elements (25 ch + 7 slack)
NFLAT = BLOC * CELLS * CHP + WIN   # per-core flat detT + tail pad

ANCHORS = np.array([1.3221, 1.73145, 3.19275, 4.00944, 5.05587,
                    8.09892, 9.47112, 4.84053, 11.2364, 10.0071],
                   dtype=np.float32)

_CACHE = {}


def _make_consts():
    """Host-precomputed, data-independent constant input tensors."""
    consts = {}
    consts["c_ident"] = np.eye(128, dtype=np.float32)
    consts["c_iota5"] = np.tile(np.arange(5, dtype=np.float32), (128, 1))
    consts["c_iota5m"] = np.tile(np.arange(5, dtype=np.float32) - 99.0, (128, 1))
    consts["c_iota20"] = np.tile(np.arange(NCLS, dtype=np.float32), (128, 1))
    consts["c_s2"] = np.tile((ANCHORS[0::2] / GRID).astype(np.float32), (128, 1))
    consts["c_s3"] = np.tile((ANCHORS[1::2] / GRID).astype(np.float32), (128, 1))
    # strict upper-triangular pair mask over (o, o2): 1.0 iff o2 > o
    tri = (np.arange(O)[None, :] > np.arange(O)[:, None]).astype(np.float32)
    consts["c_tri"] = np.tile(tri.reshape(1, O * O), (128, 1))
    # image base cell-row: object n = j*128 + p -> img = 4j + p//32
    imgo = np.empty((128, J2), dtype=np.float32)
    for p in range(128):
        for j in range(J2):
            imgo[p, j] = float(CELLS * (4 * j + p // 32))
    consts["c_imgo"] = imgo
    return consts


def _build():
    """Build the Bass module (emitted once, cached)."""
    import concourse.bacc as bacc
    import concourse.tile as tile
    from concourse import mybir
    from concourse.bass import IndirectOffsetOnAxis

    f32 = mybir.dt.float32
    i32 = mybir.dt.int32
    ALU = mybir.AluOpType
    AX = mybir.AxisListType
    ACT = mybir.ActivationFunctionType

    nc = bacc.Bacc(None, target_bir_lowering=False, debug=False)

    detF = nc.dram_tensor("detF", [NFLAT, 1], f32, kind="ExternalInput")
    conf = nc.dram_tensor("conf", [BLOC, NA, CELLS], f32,
                          kind="ExternalInput")
    gtb = nc.dram_tensor("gtb", [BLOC, O, 4], f32, kind="ExternalInput")
    clsf = nc.dram_tensor("clsf", [BLOC, O], f32, kind="ExternalInput")
    c_ident = nc.dram_tensor("c_ident", [128, 128], f32, kind="ExternalInput")
    c_iota5 = nc.dram_tensor("c_iota5", [128, 5], f32, kind="ExternalInput")
    c_iota5m = nc.dram_tensor("c_iota5m", [128, 5], f32, kind="ExternalInput")
    c_iota20 = nc.dram_tensor("c_iota20", [128, NCLS], f32,
                              kind="ExternalInput")
    c_s2 = nc.dram_tensor("c_s2", [128, 5], f32, kind="ExternalInput")
    c_s3 = nc.dram_tensor("c_s3", [128, 5], f32, kind="ExternalInput")
    c_tri = nc.dram_tensor("c_tri", [128, O * O], f32, kind="ExternalInput")
    c_imgo = nc.dram_tensor("c_imgo", [128, J2], f32, kind="ExternalInput")
    out = nc.dram_tensor("out", [128, 16], f32, kind="ExternalOutput")

    with tile.TileContext(nc) as tc:
        with tc.tile_pool(name="cpool", bufs=1) as cp, \
             tc.tile_pool(name="work", bufs=1) as wk, \
             tc.tile_pool(name="psA", bufs=2, space="PSUM") as psA:

            # ---- constants into SBUF ----
            t_id = cp.tile([128, 128], f32)
            t_i5 = cp.tile([128, 5], f32)
            t_i5m = cp.tile([128, 5], f32)
            t_i20 = cp.tile([128, NCLS], f32)
            t_s2c = cp.tile([128, 5], f32)
            t_s3c = cp.tile([128, 5], f32)
            t_tri = cp.tile([128, O * O], f32)
            t_imgo = cp.tile([128, J2], f32)
            nc.sync.dma_start(t_id[:], c_ident[:])
            nc.scalar.dma_start(t_i5[:], c_iota5[:])
            nc.scalar.dma_start(t_i5m[:], c_iota5m[:])
            nc.scalar.dma_start(t_i20[:], c_iota20[:])
            nc.scalar.dma_start(t_s2c[:], c_s2[:])
            nc.scalar.dma_start(t_s3c[:], c_s3[:])
            nc.sync.dma_start(t_tri[:], c_tri[:])
            nc.sync.dma_start(t_imgo[:], c_imgo[:])

            # ---- gt loads, object-major: object n = b*32+o = j2*128 + p ----
            # p = (b%4)*32 + o, j2 = b//4
            t_gtb = wk.tile([128, J2 * 4], f32)
            nc.sync.dma_start(
                t_gtb[:].rearrange("p (j c) -> p j c", c=4),
                gtb[:].rearrange("(j bi) o c -> (bi o) j c", bi=4))
            t_cls = wk.tile([128, J2], f32)
            nc.sync.dma_start(
                t_cls[:], clsf[:].rearrange("(j bi) o -> (bi o) j", bi=4))

            gv = t_gtb[:].rearrange("p (j c) -> p j c", c=4)
            x_ap = gv[:, :, 0]
            y_ap = gv[:, :, 1]
            w_ap = gv[:, :, 2]
            h_ap = gv[:, :, 3]

            # ---- cell coords (DVE, object-major [128, 64]) ----
            t_mx = wk.tile([128, J2], f32)
            t_my = wk.tile([128, J2], f32)
            t_tx = wk.tile([128, J2], f32)
            t_ty = wk.tile([128, J2], f32)
            t_gx = wk.tile([128, J2], f32)
            t_gy = wk.tile([128, J2], f32)
            t_k = wk.tile([128, J2], f32)
            t_scr0 = wk.tile([128, J2], f32)
            nc.vector.tensor_scalar_mul(t_mx[:], x_ap, float(GRID))
            nc.vector.tensor_scalar_mul(t_my[:], y_ap, float(GRID))
            # floor(v), robust to the fp->int rounding mode:
            #   i = cvt(v); fi = cvt_back(i); gx = fi - (fi > v)
            t_i32 = wk.tile([128, J2], i32)
            for t_m_, t_g_ in ((t_mx, t_gx), (t_my, t_gy)):
                nc.vector.tensor_copy(t_i32[:], t_m_[:])
                nc.vector.tensor_copy(t_g_[:], t_i32[:])
                nc.vector.tensor_tensor(t_scr0[:], t_g_[:], t_m_[:], ALU.is_gt)
                nc.vector.tensor_sub(t_g_[:], t_g_[:], t_scr0[:])
            nc.vector.tensor_sub(t_tx[:], t_mx[:], t_gx[:])
            nc.vector.tensor_sub(t_ty[:], t_my[:], t_gy[:])
            nc.vector.scalar_tensor_tensor(
                out=t_k[:], in0=t_gy[:], scalar=float(GRID), in1=t_gx[:],
                op0=ALU.mult, op1=ALU.add)

            # ---- round-1 offsets: element offset of object's detT row ----
            t_of1 = wk.tile([128, J2], f32)
            t_off1 = wk.tile([128, J2], i32)
            nc.vector.tensor_add(t_of1[:], t_k[:], t_imgo[:])
            nc.vector.tensor_scalar_mul(t_of1[:], t_of1[:], float(CHP))
            nc.vector.tensor_copy(t_off1[:], t_of1[:])

            # ---- round-1: 64 indirect row gathers (512B / partition) ----
            t_GT = wk.tile([128, J2 * CHP], f32)
            for j in range(J2):
                nc.gpsimd.indirect_dma_start(
                    out=t_GT[:, j * CHP:(j + 1) * CHP],
                    out_offset=None,
                    in_=detF[:],
                    in_offset=IndirectOffsetOnAxis(ap=t_off1[:, j:j + 1],
                                                   axis=0))

            # ---- dense conf: load [B,5,169] slice, square+reduce ----
            t_stage = wk.tile([128, 16], f32)
            nc.vector.memset(t_stage[:], 0.0)
            t_cf = wk.tile([128, 2 * NA * CELLS], f32)
            nc.scalar.dma_start(
                t_cf[:].rearrange("p (bh a e) -> p bh a e", a=NA, e=CELLS),
                conf[:].rearrange("(bh p) a e -> p bh a e", p=128))
            nc.vector.tensor_mul(t_cf[:], t_cf[:], t_cf[:])
            t_cfr = wk.tile([128, 2 * NA], f32)
            nc.vector.tensor_reduce(
                t_cfr[:].rearrange("p (bh a) -> p bh a", a=NA),
                t_cf[:].rearrange("p (bh a e) -> p bh a e", a=NA, e=CELLS),
                axis=AX.X, op=ALU.add)
            nc.vector.tensor_add(
                t_stage[:, 5:10], t_cfr[:, 0:NA], t_cfr[:, NA:2 * NA])

            # ---- IoU (per object x anchor) over all 64 j2 columns ----
            gtv = t_GT[:].rearrange("p (j c) -> p j c", c=CHP)
            gta = gtv[:, :, 0:NCH].rearrange("p j (a r) -> p j a r", r=CH)
            q0 = gta[:, :, :, 0]
            q1 = gta[:, :, :, 1]
            q2 = gta[:, :, :, 2]
            q3 = gta[:, :, :, 3]

            def w5(nm):
                return wk.tile([128, J2 * NA], f32, name=nm)
            t_iou = w5("t_iou"); t_scr = w5("t_scr"); t_scr2 = w5("t_scr2")
            t_pw = w5("t_pw"); t_ph = w5("t_ph")
            t_bx0 = w5("t_bx0"); t_by0 = w5("t_by0")
            t_bx1 = w5("t_bx1"); t_by1 = w5("t_by1")
            t_ix0 = w5("t_ix0"); t_iy0 = w5("t_iy0")
            t_inter = w5("t_inter"); t_den = w5("t_den")
            t_ohA = w5("t_ohA")

            def w1(nm):
                return wk.tile([128, J2], f32, name=nm)
            t_hw2 = w1("t_hw2"); t_hh2 = w1("t_hh2")
            t_gx0 = w1("t_gx0"); t_gy0 = w1("t_gy0")
            t_gx1 = w1("t_gx1"); t_gy1 = w1("t_gy1")
            t_a1 = w1("t_a1"); t_mm = w1("t_mm")
            t_aidx = w1("t_aidx"); t_sid = w1("t_sid"); t_win = w1("t_win")
            t_s2w = w1("t_s2w"); t_s3w = w1("t_s3w")
            s_csse = w1("s_csse"); s_c1 = w1("s_c1")
            s_q4 = w1("s_q4"); s_cls = w1("s_cls")

            def r5(t):
                return t[:].rearrange("p (j a) -> p j a", a=NA)

            def b5(ap2d):  # [128, J2] -> broadcast [128, J2, 5]
                return ap2d.rearrange("p (j one) -> p j one", one=1) \
                           .to_broadcast([128, J2, NA])

            def c5(tile1):  # const [128, 5] -> [128, J2, 5]
                return tile1[:].rearrange("p (one a) -> p one a", one=1) \
                               .to_broadcast([128, J2, NA])

            nc.vector.tensor_tensor(r5(t_pw), q2, c5(t_s2c), ALU.mult)
            nc.vector.tensor_tensor(r5(t_ph), q3, c5(t_s3c), ALU.mult)
            # bx0 = (px+gx)/13 - pw/2 ; by0 = (py+gy)/13 - ph/2
            nc.vector.tensor_tensor(r5(t_bx0), q0, b5(t_gx[:]), ALU.add)
            nc.vector.tensor_scalar_mul(t_bx0[:], t_bx0[:], 1.0 / GRID)
            nc.vector.scalar_tensor_tensor(
                out=t_bx0[:], in0=t_pw[:], scalar=-0.5, in1=t_bx0[:],
                op0=ALU.mult, op1=ALU.add)
            nc.vector.tensor_tensor(r5(t_by0), q1, b5(t_gy[:]), ALU.add)
            nc.vector.tensor_scalar_mul(t_by0[:], t_by0[:], 1.0 / GRID)
            nc.vector.scalar_tensor_tensor(
                out=t_by0[:], in0=t_ph[:], scalar=-0.5, in1=t_by0[:],
                op0=ALU.mult, op1=ALU.add)
            nc.vector.tensor_add(t_bx1[:], t_bx0[:], t_pw[:])
            nc.vector.tensor_add(t_by1[:], t_by0[:], t_ph[:])
            # gt box corners [128, J2]
            nc.vector.tensor_scalar_mul(t_hw2[:], w_ap, 0.5)
            nc.vector.tensor_scalar_mul(t_hh2[:], h_ap, 0.5)
            nc.vector.tensor_sub(t_gx0[:], x_ap, t_hw2[:])
            nc.vector.tensor_add(t_gx1[:], x_ap, t_hw2[:])
            nc.vector.tensor_sub(t_gy0[:], y_ap, t_hh2[:])
            nc.vector.tensor_add(t_gy1[:], y_ap, t_hh2[:])
            # a1 = (gx1-gx0+1)*(gy1-gy0+1)
            nc.vector.tensor_sub(t_a1[:], t_gx1[:], t_gx0[:])
            nc.vector.tensor_scalar_add(t_a1[:], t_a1[:], 1.0)
            nc.vector.tensor_sub(t_mm[:], t_gy1[:], t_gy0[:])
            nc.vector.tensor_scalar_add(t_mm[:], t_mm[:], 1.0)
            nc.vector.tensor_mul(t_a1[:], t_a1[:], t_mm[:])
            # intersection: ix0 = max(gx0,bx0); ix1 = min(gx1,bx1) (in bx1)
            nc.vector.tensor_tensor(r5(t_ix0), r5(t_bx0), b5(t_gx0[:]), ALU.max)
            nc.vector.tensor_tensor(r5(t_iy0), r5(t_by0), b5(t_gy0[:]), ALU.max)
            nc.vector.tensor_tensor(r5(t_bx1), r5(t_bx1), b5(t_gx1[:]), ALU.min)
            nc.vector.tensor_tensor(r5(t_by1), r5(t_by1), b5(t_gy1[:]), ALU.min)
            nc.vector.tensor_sub(t_bx1[:], t_bx1[:], t_ix0[:])
            nc.vector.tensor_scalar_add(t_bx1[:], t_bx1[:], 1.0)
            nc.vector.tensor_sub(t_by1[:], t_by1[:], t_iy0[:])
            nc.vector.tensor_scalar_add(t_by1[:], t_by1[:], 1.0)
            nc.vector.tensor_mul(t_inter[:], t_bx1[:], t_by1[:])
            # a2 = (pw+1)*(ph+1); denom = a1 + a2 - inter
            nc.vector.tensor_scalar_add(t_pw[:], t_pw[:], 1.0)
            nc.vector.tensor_scalar_add(t_ph[:], t_ph[:], 1.0)
            nc.vector.tensor_mul(t_den[:], t_pw[:], t_ph[:])
            nc.vector.tensor_tensor(r5(t_den), r5(t_den), b5(t_a1[:]), ALU.add)
            nc.vector.tensor_sub(t_den[:], t_den[:], t_inter[:])
            nc.vector.reciprocal(t_den[:], t_den[:])
            nc.vector.tensor_mul(t_iou[:], t_inter[:], t_den[:])

            # ---- argmax over anchors (first max wins) ----
            nc.vector.reduce_max(t_mm[:], r5(t_iou), axis=AX.X)
            nc.vector.tensor_tensor(
                r5(t_scr), r5(t_iou), b5(t_mm[:]), ALU.is_equal)
            nc.vector.tensor_tensor(
                r5(t_scr2), r5(t_scr), c5(t_i5m), ALU.mult)
            nc.vector.tensor_reduce(
                t_aidx[:], r5(t_scr2), axis=AX.X, op=ALU.min)
            nc.vector.tensor_scalar_add(t_aidx[:], t_aidx[:], 99.0)

            # ---- round-2 offsets: winner sub-row = row*128 + 25*aidx ----
            t_off2 = wk.tile([128, J2], i32)
            nc.vector.scalar_tensor_tensor(
                out=t_scr0[:], in0=t_aidx[:], scalar=float(CH), in1=t_of1[:],
                op0=ALU.mult, op1=ALU.add)
            nc.vector.tensor_copy(t_off2[:], t_scr0[:])

            # ---- round-2: 64 indirect winner gathers (128B / partition) ----
            t_W = wk.tile([128, J2 * WIN], f32)
            for j in range(J2):
                nc.gpsimd.indirect_dma_start(
                    out=t_W[:, j * WIN:(j + 1) * WIN],
                    out_offset=None,
                    in_=detF[:],
                    in_offset=IndirectOffsetOnAxis(ap=t_off2[:, j:j + 1],
                                                   axis=0))

            # ---- slot id + single-shot dedup (overlaps round-2 DMAs) ----
            nc.vector.scalar_tensor_tensor(
                out=t_sid[:], in0=t_aidx[:], scalar=float(CELLS),
                in1=t_k[:], op0=ALU.mult, op1=ALU.add)
            t_sT = wk.tile([J2, 128], f32)
            t_eqp = wk.tile([J2, 4 * O * O], f32)
            t_deadT = wk.tile([J2, 128], f32)
            t_tp1 = psA.tile([J2, 128], f32, space="PSUM", tag="ded", bufs=1)
            nc.tensor.transpose(out=t_tp1[:], in_=t_sid[:], identity=t_id[:])
            nc.scalar.activation(t_sT[:], t_tp1[:], ACT.Copy)
            sTa = t_sT[:].rearrange("p (i o one) -> p i o one", i=4, one=1) \
                         .to_broadcast([J2, 4, O, O])
            sTb = t_sT[:].rearrange("p (i one o2) -> p i one o2", i=4, one=1) \
                         .to_broadcast([J2, 4, O, O])
            eqv = t_eqp[:].rearrange("p (i o o2) -> p i o o2", i=4, o2=O)
            nc.vector.tensor_tensor(eqv, sTa, sTb, ALU.is_equal)
            triv = t_tri[0:J2, :].rearrange(
                "p (one o o2) -> p one o o2", one=1, o2=O) \
                .to_broadcast([J2, 4, O, O])
            nc.vector.tensor_tensor(eqv, eqv, triv, ALU.mult)
            nc.vector.tensor_reduce(
                t_deadT[:].rearrange("p (i o) -> p i o", o=O),
                eqv, axis=AX.X, op=ALU.max)
            t_tp2 = psA.tile([128, J2], f32, space="PSUM", tag="ded2", bufs=1)
            nc.tensor.transpose(
                out=t_tp2[:], in_=t_deadT[:], identity=t_id[0:J2, 0:J2])
            nc.scalar.activation(t_win[:], t_tp2[:], ACT.Copy)
            nc.vector.tensor_scalar(
                t_win[:], t_win[:], -1.0, 1.0, ALU.mult, ALU.add)

            # ---- winner-anchor scales: s2w = anchors[2a]/13 etc. ----
            nc.vector.tensor_tensor(
                r5(t_ohA), b5(t_aidx[:]), c5(t_i5), ALU.is_equal)
            nc.vector.tensor_tensor(r5(t_scr), r5(t_ohA), c5(t_s2c), ALU.mult)
            nc.vector.tensor_reduce(t_s2w[:], r5(t_scr), axis=AX.X, op=ALU.add)
            nc.vector.tensor_tensor(r5(t_scr), r5(t_ohA), c5(t_s3c), ALU.mult)
            nc.vector.tensor_reduce(t_s3w[:], r5(t_scr), axis=AX.X, op=ALU.add)

            # ---- winner terms from t_W [128, 64, 32] ----
            wv = t_W[:].rearrange("p (j c) -> p j c", c=WIN)
            t_d4 = wk.tile([128, J2 * 4], f32)
            d4 = t_d4[:].rearrange("p (j c) -> p j c", c=4)
            nc.vector.tensor_tensor(d4[:, :, 0:1], wv[:, :, 0:1],
                                    t_tx[:].rearrange("p (j one) -> p j one",
                                                      one=1), ALU.subtract)
            nc.vector.tensor_tensor(d4[:, :, 1:2], wv[:, :, 1:2],
                                    t_ty[:].rearrange("p (j one) -> p j one",
                                                      one=1), ALU.subtract)
            nc.vector.tensor_mul(t_scr0[:], wv[:, :, 2], t_s2w[:])
            nc.vector.tensor_sub(d4[:, :, 2], t_scr0[:], w_ap)
            nc.vector.tensor_mul(t_scr0[:], wv[:, :, 3], t_s3w[:])
            nc.vector.tensor_sub(d4[:, :, 3], t_scr0[:], h_ap)
            nc.vector.tensor_mul(t_d4[:], t_d4[:], t_d4[:])
            nc.vector.tensor_reduce(s_csse[:], d4, axis=AX.X, op=ALU.add)

            # conf terms at winner: (1-q4)^2 and q4^2
            nc.vector.tensor_scalar(
                t_scr0[:], wv[:, :, 4], -1.0, 1.0, ALU.mult, ALU.add)
            nc.vector.tensor_mul(s_c1[:], t_scr0[:], t_scr0[:])
            nc.vector.tensor_mul(s_q4[:], wv[:, :, 4], wv[:, :, 4])

            # class terms at winner: S2 - 2*qcls
            t_oh = wk.tile([128, J2 * NCLS], f32)
            t_t20 = wk.tile([128, J2 * NCLS], f32)
            ohv = t_oh[:].rearrange("p (j c) -> p j c", c=NCLS)
            nc.vector.tensor_tensor(
                ohv,
                t_cls[:].rearrange("p (j one) -> p j one", one=1)
                .to_broadcast([128, J2, NCLS]),
                t_i20[:].rearrange("p (one c) -> p one c", one=1)
                .to_broadcast([128, J2, NCLS]),
                ALU.is_equal)
            t20v = t_t20[:].rearrange("p (j c) -> p j c", c=NCLS)
            nc.vector.tensor_tensor(t20v, wv[:, :, 5:CH], ohv, ALU.mult)
            nc.vector.tensor_reduce(s_cls[:], t20v, axis=AX.X, op=ALU.add)
            nc.vector.tensor_tensor(
                t20v, wv[:, :, 5:CH], wv[:, :, 5:CH], ALU.mult)
            nc.vector.tensor_reduce(t_scr0[:], t20v, axis=AX.X, op=ALU.add)
            # s_cls = S2 - 2*qcls  (the +1 handled via sum(win))
            nc.vector.scalar_tensor_tensor(
                out=s_cls[:], in0=s_cls[:], scalar=-2.0, in1=t_scr0[:],
                op0=ALU.mult, op1=ALU.add)

            # ---- win-masked partial sums into staging ----
            t_red1 = wk.tile([128, 1], f32)

            def accw(col, stash):
                nc.vector.tensor_mul(stash[:], stash[:], t_win[:])
                nc.vector.reduce_sum(t_red1[:], stash[:], axis=AX.X)
                nc.vector.tensor_copy(t_stage[:, col:col + 1], t_red1[:])
            accw(0, s_csse)   # coord SSE (unweighted by 5)
            accw(1, s_c1)     # (1-q4)^2 at slots
            accw(2, s_q4)     # q4^2 at slots
            accw(3, s_cls)    # S2 - 2*qcls at slots
            nc.vector.reduce_sum(t_red1[:], t_win[:], axis=AX.X)
            nc.vector.tensor_copy(t_stage[:, 4:5], t_red1[:])

            nc.sync.dma_start(out[:], t_stage[:])

    nc.compile()
    return nc


def _get_built():
    if "nc" not in _CACHE:
        _CACHE["nc"] = _build()
        _CACHE["consts"] = _make_consts()
    return _CACHE["nc"], _CACHE["consts"]


def _reduce_partials(P):
    """P: [ncores, 128, 16] fp32 partials -> the 4 scalar losses."""
    S = P.astype(np.float64).sum(axis=(0, 1))
    coord, confobj, confsub, clsq, wsum = S[0], S[1], S[2], S[3], S[4]
    dense = S[5:10].sum()
    obj_loss = 5.0 * coord + confobj
    no_obj_loss = 0.5 * (dense - confsub)
    conf_loss = clsq + wsum
    loss = obj_loss + no_obj_loss + conf_loss
    return (np.float32(loss), np.float32(obj_loss),
            np.float32(no_obj_loss), np.float32(conf_loss))


def kernel(detection_result, gt_boxes, gt_class):
    from concourse.bass_utils import run_bass_kernel_spmd

    nc, consts = _get_built()
    det = np.ascontiguousarray(
        np.asarray(detection_result, dtype=np.float32)).reshape(B, NCH, CELLS)
    # cell-major, channel-padded flat copy for the indirect row gathers
    flat = np.zeros(B * CELLS * CHP + WIN, dtype=np.float32)
    flat[:B * CELLS * CHP].reshape(B, CELLS, CHP)[:, :, :NCH] = \
        det.transpose(0, 2, 1)
    confs = np.ascontiguousarray(det[:, 4::CH, :])  # [B, 5, 169]
    gtb = np.ascontiguousarray(np.asarray(gt_boxes, dtype=np.float32))
    clsf = np.asarray(gt_class).astype(np.float32)

    STRIDE = BLOC * CELLS * CHP
    in_maps = []
    for c in range(NCORES):
        sl = slice(c * BLOC, (c + 1) * BLOC)
        m = {"detF": flat[c * STRIDE:(c + 1) * STRIDE + WIN].reshape(-1, 1),
             "conf": confs[sl], "gtb": gtb[sl], "clsf": clsf[sl]}
        m.update(consts)
        in_maps.append(m)

    res = run_bass_kernel_spmd(nc, in_maps, core_ids=list(range(NCORES)))
    _CACHE["last_res"] = res
    P = np.stack([res.results[c]["out"] for c in range(NCORES)])
    return _reduce_partials(P)


# revision 16
# speedup vs baseline: 2.4325x; 1.3859x over previous
"""Trainium2 Bass kernel for nn_DetectionLoss (YOLO-style detection loss).

Strategy (pure data parallel over 8 NeuronCores, 256 images each):
  - Host reformats det [B,125,13,13] into cell-major detT rows
    [169*B, 128] (zero-padded ch) plus a compact conf slice [B,5,169].
  - The loss reads only ~23% of det: per core, 8 hardware dma_gather
    calls (SWDGE, 1024 rows each, 512B contiguous per row) pull the 8192
    object rows straight from HBM into an object-major [128, 64*128]
    tile.  One dense 865KB load covers the no-object conf term.
    ~5MB HBM/core vs 22.5MB dense.
  - The descriptor generation (~8.6us per call on the Pool engine) is the
    backbone; the DVE IoU / argmax / loss math runs in 4 passes of 16
    object-columns pipelined under it.  A single-shot pairwise
    last-writer-wins dedup on [64, 4096] plus the masked sums form the
    tail.
  - Output: per-core partial sums [128, 16]; host reduces across cores.
"""
import numpy as np

GRID = 13
NA = 5
NCLS = 20
CH = 25
NCH = NA * CH          # 125
CHP = 128              # padded channel dim in detT rows
CELLS = GRID * GRID    # 169
O = 32                 # objects per image
B = 2048               # global batch
NCORES = 8
BLOC = B // NCORES     # 256 images per core
NOBJ = BLOC * O        # 8192 objects per core
J2 = NOBJ // 128       # 64 object columns
NCALL = 8              # gather calls (hardware limit ~1024 idxs per call)
IPC = BLOC // NCALL    # 32 images per call
NIC = IPC * O          # 1024 idxs per call
JC = NIC // 128        # 8 j2-columns per call
NPASS = 4
JPP = J2 // NPASS      # 16 j2-columns per pass (2 gather calls)

ANCHORS = np.array([1.3221, 1.73145, 3.19275, 4.00944, 5.05587,
                    8.09892, 9.47112, 4.84053, 11.2364, 10.0071],
                   dtype=np.float32)

_CACHE = {}


def _make_consts():
    """Host-precomputed, data-independent constant input tensors."""
    consts = {}
    consts["c_ident"] = np.eye(128, dtype=np.float32)
    # 8 partition-selector matrices for the idx shuffle, packed [128, 8*128].
    # matmul r: out_r[i, :] = k_obj[q, :] with q = (r//2)*32 + (r%2)*16 + i%16
    sel = np.zeros((128, 8 * 128), dtype=np.float32)
    for r in range(8):
        for i in range(128):
            sel[(r // 2) * 32 + (r % 2) * 16 + (i % 16), r * 128 + i] = 1.0
    consts["c_sel"] = sel
    consts["c_iota5"] = np.tile(np.arange(5, dtype=np.float32), (128, 1))
    consts["c_iota5m"] = np.tile(np.arange(5, dtype=np.float32) - 99.0, (128, 1))
    consts["c_iota20"] = np.tile(np.arange(NCLS, dtype=np.float32), (128, 1))
    consts["c_s2"] = np.tile((ANCHORS[0::2] / GRID).astype(np.float32), (128, 1))
    consts["c_s3"] = np.tile((ANCHORS[1::2] / GRID).astype(np.float32), (128, 1))
    # strict upper-triangular pair mask over (o, o2): 1.0 iff o2 > o
    tri = (np.arange(O)[None, :] > np.arange(O)[:, None]).astype(np.float32)
    consts["c_tri"] = np.tile(tri.reshape(1, O * O), (128, 1))
    # imgbase[p, cq*64 + m*8 + r] = 169 * (4m + r//2)  (img_local of idx slot)
    ib = np.zeros((128, NCALL * 64), dtype=np.float32)
    for cq in range(NCALL):
        for m in range(8):
            for r in range(8):
                ib[:, cq * 64 + m * 8 + r] = float(CELLS * (4 * m + r // 2))
    consts["c_ib"] = ib
    return consts


def _build():
    """Build the Bass module (emitted once, cached)."""
    import concourse.bacc as bacc
    import concourse.tile as tile
    from concourse import mybir

    f32 = mybir.dt.float32
    i16 = mybir.dt.int16
    i32 = mybir.dt.int32
    ALU = mybir.AluOpType
    AX = mybir.AxisListType
    ACT = mybir.ActivationFunctionType

    nc = bacc.Bacc(None, target_bir_lowering=False, debug=False)

    detT = nc.dram_tensor("detT", [BLOC * CELLS, CHP], f32,
                          kind="ExternalInput")
    conf = nc.dram_tensor("conf", [BLOC, NA, CELLS], f32,
                          kind="ExternalInput")
    gtb = nc.dram_tensor("gtb", [BLOC, O, 4], f32, kind="ExternalInput")
    clsf = nc.dram_tensor("clsf", [BLOC, O], f32, kind="ExternalInput")
    c_ident = nc.dram_tensor("c_ident", [128, 128], f32, kind="ExternalInput")
    c_sel = nc.dram_tensor("c_sel", [128, 8 * 128], f32, kind="ExternalInput")
    c_iota5 = nc.dram_tensor("c_iota5", [128, 5], f32, kind="ExternalInput")
    c_iota5m = nc.dram_tensor("c_iota5m", [128, 5], f32, kind="ExternalInput")
    c_iota20 = nc.dram_tensor("c_iota20", [128, NCLS], f32,
                              kind="ExternalInput")
    c_s2 = nc.dram_tensor("c_s2", [128, 5], f32, kind="ExternalInput")
    c_s3 = nc.dram_tensor("c_s3", [128, 5], f32, kind="ExternalInput")
    c_tri = nc.dram_tensor("c_tri", [128, O * O], f32, kind="ExternalInput")
    c_ib = nc.dram_tensor("c_ib", [128, NCALL * 64], f32,
                          kind="ExternalInput")
    out = nc.dram_tensor("out", [128, 16], f32, kind="ExternalOutput")

    with tile.TileContext(nc) as tc:
        with tc.tile_pool(name="cpool", bufs=1) as cp, \
             tc.tile_pool(name="work", bufs=1) as wk, \
             tc.tile_pool(name="psA", bufs=2, space="PSUM") as psA:

            # ---- constants into SBUF (split across sync/scalar queues) ----
            t_id = cp.tile([128, 128], f32)
            t_sel = cp.tile([128, 8 * 128], f32)
            t_i5 = cp.tile([128, 5], f32)
            t_i5m = cp.tile([128, 5], f32)
            t_i20 = cp.tile([128, NCLS], f32)
            t_s2c = cp.tile([128, 5], f32)
            t_s3c = cp.tile([128, 5], f32)
            t_tri = cp.tile([128, O * O], f32)
            t_ib = cp.tile([128, NCALL * 64], f32)
            nc.sync.dma_start(t_id[:], c_ident[:])
            nc.sync.dma_start(t_sel[:], c_sel[:])
            nc.scalar.dma_start(t_i5[:], c_iota5[:])
            nc.scalar.dma_start(t_i5m[:], c_iota5m[:])
            nc.scalar.dma_start(t_i20[:], c_iota20[:])
            nc.scalar.dma_start(t_s2c[:], c_s2[:])
            nc.scalar.dma_start(t_s3c[:], c_s3[:])
            nc.scalar.dma_start(t_tri[:], c_tri[:])
            nc.sync.dma_start(t_ib[:], c_ib[:])

            # ---- gt loads, object-major: object n = b*32+o = j2*128 + p ----
            # p = (b%4)*32 + o, j2 = b//4
            t_gtb = wk.tile([128, J2 * 4], f32)
            nc.sync.dma_start(
                t_gtb[:].rearrange("p (j c) -> p j c", c=4),
                gtb[:].rearrange("(j bi) o c -> (bi o) j c", bi=4))
            t_cls = wk.tile([128, J2], f32)
            nc.sync.dma_start(
                t_cls[:], clsf[:].rearrange("(j bi) o -> (bi o) j", bi=4))

            gv = t_gtb[:].rearrange("p (j c) -> p j c", c=4)
            x_ap = gv[:, :, 0]
            y_ap = gv[:, :, 1]
            w_ap = gv[:, :, 2]
            h_ap = gv[:, :, 3]

            # ---- cell coords (DVE, object-major [128, 64]) ----
            t_mx = wk.tile([128, J2], f32)
            t_my = wk.tile([128, J2], f32)
            t_tx = wk.tile([128, J2], f32)
            t_ty = wk.tile([128, J2], f32)
            t_gx = wk.tile([128, J2], f32)
            t_gy = wk.tile([128, J2], f32)
            t_k = wk.tile([128, J2], f32)
            t_scr0 = wk.tile([128, J2], f32)
            nc.vector.tensor_scalar_mul(t_mx[:], x_ap, float(GRID))
            nc.vector.tensor_scalar_mul(t_my[:], y_ap, float(GRID))
            # floor(v), robust to the fp->int rounding mode:
            #   i = cvt(v); fi = cvt_back(i); gx = fi - (fi > v)
            t_i32 = wk.tile([128, J2], i32)
            for t_m_, t_g_ in ((t_mx, t_gx), (t_my, t_gy)):
                nc.vector.tensor_copy(t_i32[:], t_m_[:])
                nc.vector.tensor_copy(t_g_[:], t_i32[:])
                nc.vector.tensor_tensor(t_scr0[:], t_g_[:], t_m_[:], ALU.is_gt)
                nc.vector.tensor_sub(t_g_[:], t_g_[:], t_scr0[:])
            nc.vector.tensor_sub(t_tx[:], t_mx[:], t_gx[:])
            nc.vector.tensor_sub(t_ty[:], t_my[:], t_gy[:])
            nc.vector.scalar_tensor_tensor(
                out=t_k[:], in0=t_gy[:], scalar=float(GRID), in1=t_gx[:],
                op0=ALU.mult, op1=ALU.add)

            # ---- gather-index shuffle into dma_gather's wrapped layout ----
            # call cq covers objects n in [1024cq, 1024cq+1024); position
            # i = n%1024 consumed at idxs[p16=i%16, jcol=i//16].  idxs value
            # = 169*img_local + k[n];  source t_k[q=(jcol%8)*16+p16, j2].
            # Selector r: out_r[i, j2] = k[(r//2)*32+(r%2)*16+i%16, j2]
            t_idxf = wk.tile([128, NCALL * 64], f32)
            for r in range(8):
                t_pr = psA.tile([128, J2], f32, space="PSUM", tag="shuf")
                nc.tensor.matmul(
                    out=t_pr[:], lhsT=t_sel[:, r * 128:(r + 1) * 128],
                    rhs=t_k[:], start=True, stop=True)
                nc.scalar.activation(
                    t_idxf[:].rearrange("p (cq m r) -> p cq m r", cq=NCALL,
                                        r=8)[:, :, :, r],
                    t_pr[:].rearrange("p (cq m) -> p cq m", cq=NCALL),
                    ACT.Copy)
            t_idx16 = wk.tile([128, NCALL * 64], i16)
            nc.vector.tensor_add(t_idxf[:], t_idxf[:], t_ib[:])
            nc.vector.tensor_copy(t_idx16[:], t_idxf[:])

            # ---- object-row gathers straight from HBM (hardware SWDGE) ----
            t_GT = wk.tile([128, J2 * CHP], f32)
            for cq in range(NCALL):
                nc.gpsimd.dma_gather(
                    out_ap=t_GT[:, cq * JC * CHP:(cq + 1) * JC * CHP]
                    .rearrange("p (g c) -> p g c", c=CHP),
                    in_ap=detT[cq * IPC * CELLS:(cq + 1) * IPC * CELLS],
                    idxs_ap=t_idx16[:, cq * 64:(cq + 1) * 64],
                    num_idxs=NIC, num_idxs_reg=NIC, elem_size=CHP)

            # ---- dense conf: load [B,5,169] slice, square+reduce ----
            t_stage = wk.tile([128, 16], f32)
            nc.vector.memset(t_stage[:], 0.0)
            t_cf = wk.tile([128, 2 * NA * CELLS], f32)
            nc.scalar.dma_start(
                t_cf[:].rearrange("p (bh a e) -> p bh a e", a=NA, e=CELLS),
                conf[:].rearrange("(bh p) a e -> p bh a e", p=128))
            nc.vector.tensor_mul(t_cf[:], t_cf[:], t_cf[:])
            t_cfr = wk.tile([128, 2 * NA], f32)
            nc.vector.tensor_reduce(
                t_cfr[:].rearrange("p (bh a) -> p bh a", a=NA),
                t_cf[:].rearrange("p (bh a e) -> p bh a e", a=NA, e=CELLS),
                axis=AX.X, op=ALU.add)
            nc.vector.tensor_add(
                t_stage[:, 5:10], t_cfr[:, 0:NA], t_cfr[:, NA:2 * NA])

            # ---- per-pass work tiles (shared across the 4 passes) ----
            def w5(nm):
                return wk.tile([128, JPP * NA], f32, name=nm)
            t_iou = w5("t_iou"); t_scr = w5("t_scr"); t_scr2 = w5("t_scr2")
            t_pw = w5("t_pw"); t_ph = w5("t_ph")
            t_bx0 = w5("t_bx0"); t_by0 = w5("t_by0")
            t_bx1 = w5("t_bx1"); t_by1 = w5("t_by1")
            t_ix0 = w5("t_ix0"); t_iy0 = w5("t_iy0")
            t_inter = w5("t_inter"); t_den = w5("t_den")
            t_ohA = w5("t_ohA")

            def wp(nm):
                return wk.tile([128, JPP], f32, name=nm)
            t_hw2 = wp("t_hw2"); t_hh2 = wp("t_hh2")
            t_gx0 = wp("t_gx0"); t_gy0 = wp("t_gy0")
            t_gx1 = wp("t_gx1"); t_gy1 = wp("t_gy1")
            t_a1 = wp("t_a1"); t_mm = wp("t_mm"); t_aidx = wp("t_aidx")
            t_oh = wk.tile([128, JPP * NCLS], f32)
            t_t20 = wk.tile([128, JPP * NCLS], f32)
            t_qcl = wk.tile([128, JPP * NA], f32, name="t_qcl")
            t_diff = wk.tile([128, JPP * NA * 4], f32)

            def w1(nm):
                return wk.tile([128, J2], f32, name=nm)
            t_sid = w1("t_sid"); t_win = w1("t_win")
            s_csse = w1("s_csse"); s_c1 = w1("s_c1")
            s_q4 = w1("s_q4"); s_cls = w1("s_cls")

            def r5(t):
                return t[:].rearrange("p (j a) -> p j a", a=NA)

            def b5(ap2d):  # [128, JPP] -> broadcast [128, JPP, 5]
                return ap2d.rearrange("p (j one) -> p j one", one=1) \
                           .to_broadcast([128, JPP, NA])

            def c5(tile1):  # const [128, 5] -> [128, JPP, 5]
                return tile1[:].rearrange("p (one a) -> p one a", one=1) \
                               .to_broadcast([128, JPP, NA])

            gtv = t_GT[:].rearrange("p (j c) -> p j c", c=CHP)

            for ps in range(NPASS):
                jsl = slice(ps * JPP, (ps + 1) * JPP)
                gp = gtv[:, jsl, 0:NCH].rearrange("p j (a r) -> p j a r", r=CH)
                q0 = gp[:, :, :, 0]
                q1 = gp[:, :, :, 1]
                q2 = gp[:, :, :, 2]
                q3 = gp[:, :, :, 3]
                q4 = gp[:, :, :, 4]
                qclsv = gp[:, :, :, 5:CH]          # [p, JPP, a, 20]

                # ---- IoU (per object x anchor) ----
                nc.vector.tensor_tensor(r5(t_pw), q2, c5(t_s2c), ALU.mult)
                nc.vector.tensor_tensor(r5(t_ph), q3, c5(t_s3c), ALU.mult)
                nc.vector.tensor_tensor(r5(t_bx0), q0, b5(t_gx[:, jsl]),
                                        ALU.add)
                nc.vector.tensor_scalar_mul(t_bx0[:], t_bx0[:], 1.0 / GRID)
                nc.vector.scalar_tensor_tensor(
                    out=t_bx0[:], in0=t_pw[:], scalar=-0.5, in1=t_bx0[:],
                    op0=ALU.mult, op1=ALU.add)
                nc.vector.tensor_tensor(r5(t_by0), q1, b5(t_gy[:, jsl]),
                                        ALU.add)
                nc.vector.tensor_scalar_mul(t_by0[:], t_by0[:], 1.0 / GRID)
                nc.vector.scalar_tensor_tensor(
                    out=t_by0[:], in0=t_ph[:], scalar=-0.5, in1=t_by0[:],
                    op0=ALU.mult, op1=ALU.add)
                nc.vector.tensor_add(t_bx1[:], t_bx0[:], t_pw[:])
                nc.vector.tensor_add(t_by1[:], t_by0[:], t_ph[:])
                nc.vector.tensor_scalar_mul(t_hw2[:], w_ap[:, jsl], 0.5)
                nc.vector.tensor_scalar_mul(t_hh2[:], h_ap[:, jsl], 0.5)
                nc.vector.tensor_sub(t_gx0[:], x_ap[:, jsl], t_hw2[:])
                nc.vector.tensor_add(t_gx1[:], x_ap[:, jsl], t_hw2[:])
                nc.vector.tensor_sub(t_gy0[:], y_ap[:, jsl], t_hh2[:])
                nc.vector.tensor_add(t_gy1[:], y_ap[:, jsl], t_hh2[:])
                nc.vector.tensor_sub(t_a1[:], t_gx1[:], t_gx0[:])
                nc.vector.tensor_scalar_add(t_a1[:], t_a1[:], 1.0)
                nc.vector.tensor_sub(t_mm[:], t_gy1[:], t_gy0[:])
                nc.vector.tensor_scalar_add(t_mm[:], t_mm[:], 1.0)
                nc.vector.tensor_mul(t_a1[:], t_a1[:], t_mm[:])
                nc.vector.tensor_tensor(r5(t_ix0), r5(t_bx0), b5(t_gx0[:]),
                                        ALU.max)
                nc.vector.tensor_tensor(r5(t_iy0), r5(t_by0), b5(t_gy0[:]),
                                        ALU.max)
                nc.vector.tensor_tensor(r5(t_bx1), r5(t_bx1), b5(t_gx1[:]),
                                        ALU.min)
                nc.vector.tensor_tensor(r5(t_by1), r5(t_by1), b5(t_gy1[:]),
                                        ALU.min)
                nc.vector.tensor_sub(t_bx1[:], t_bx1[:], t_ix0[:])
                nc.vector.tensor_scalar_add(t_bx1[:], t_bx1[:], 1.0)
                nc.vector.tensor_sub(t_by1[:], t_by1[:], t_iy0[:])
                nc.vector.tensor_scalar_add(t_by1[:], t_by1[:], 1.0)
                nc.vector.tensor_mul(t_inter[:], t_bx1[:], t_by1[:])
                nc.vector.tensor_scalar_add(t_pw[:], t_pw[:], 1.0)
                nc.vector.tensor_scalar_add(t_ph[:], t_ph[:], 1.0)
                nc.vector.tensor_mul(t_den[:], t_pw[:], t_ph[:])
                nc.vector.tensor_tensor(r5(t_den), r5(t_den), b5(t_a1[:]),
                                        ALU.add)
                nc.vector.tensor_sub(t_den[:], t_den[:], t_inter[:])
                nc.vector.reciprocal(t_den[:], t_den[:])
                nc.vector.tensor_mul(t_iou[:], t_inter[:], t_den[:])

                # ---- argmax over anchors (first max wins) ----
                nc.vector.reduce_max(t_mm[:], r5(t_iou), axis=AX.X)
                nc.vector.tensor_tensor(
                    r5(t_scr), r5(t_iou), b5(t_mm[:]), ALU.is_equal)
                nc.vector.tensor_tensor(
                    r5(t_scr2), r5(t_scr), c5(t_i5m), ALU.mult)
                nc.vector.tensor_reduce(
                    t_aidx[:], r5(t_scr2), axis=AX.X, op=ALU.min)
                nc.vector.tensor_scalar_add(t_aidx[:], t_aidx[:], 99.0)

                # ---- slot id s = 169*aidx + k ----
                nc.vector.scalar_tensor_tensor(
                    out=t_sid[:, jsl], in0=t_aidx[:], scalar=float(CELLS),
                    in1=t_k[:, jsl], op0=ALU.mult, op1=ALU.add)

                # ---- masks: onehot(aidx) ----
                nc.vector.tensor_tensor(
                    r5(t_ohA), b5(t_aidx[:]), c5(t_i5), ALU.is_equal)

                # ---- coord SSE, anchor-reduced via onehot ----
                dv = t_diff[:].rearrange("p (j a c) -> p j a c", a=NA, c=4)
                nc.vector.tensor_tensor(
                    dv[:, :, :, 0], q0, b5(t_tx[:, jsl]), ALU.subtract)
                nc.vector.tensor_tensor(
                    dv[:, :, :, 1], q1, b5(t_ty[:, jsl]), ALU.subtract)
                nc.vector.tensor_tensor(r5(t_scr), q2, c5(t_s2c), ALU.mult)
                nc.vector.tensor_tensor(
                    dv[:, :, :, 2], r5(t_scr), b5(w_ap[:, jsl]), ALU.subtract)
                nc.vector.tensor_tensor(r5(t_scr), q3, c5(t_s3c), ALU.mult)
                nc.vector.tensor_tensor(
                    dv[:, :, :, 3], r5(t_scr), b5(h_ap[:, jsl]), ALU.subtract)
                nc.vector.tensor_mul(t_diff[:], t_diff[:], t_diff[:])
                nc.vector.tensor_reduce(r5(t_scr2), dv, axis=AX.X, op=ALU.add)
                nc.vector.tensor_mul(t_scr2[:], t_scr2[:], t_ohA[:])
                nc.vector.tensor_reduce(s_csse[:, jsl], r5(t_scr2), axis=AX.X,
                                        op=ALU.add)

                # ---- conf terms: (1-q4)^2 and q4^2, anchor-reduced ----
                nc.vector.tensor_scalar(
                    r5(t_scr), q4, -1.0, 1.0, ALU.mult, ALU.add)
                nc.vector.tensor_mul(t_scr[:], t_scr[:], t_scr[:])
                nc.vector.tensor_mul(t_scr[:], t_scr[:], t_ohA[:])
                nc.vector.tensor_reduce(s_c1[:, jsl], r5(t_scr), axis=AX.X,
                                        op=ALU.add)
                nc.vector.tensor_tensor(r5(t_scr), q4, q4, ALU.mult)
                nc.vector.tensor_mul(t_scr[:], t_scr[:], t_ohA[:])
                nc.vector.tensor_reduce(s_q4[:, jsl], r5(t_scr), axis=AX.X,
                                        op=ALU.add)

                # ---- class terms: S2 - 2*qcls at winner anchor ----
                ohv = t_oh[:].rearrange("p (j c) -> p j c", c=NCLS)
                nc.vector.tensor_tensor(
                    ohv,
                    t_cls[:, jsl].rearrange("p (j one) -> p j one", one=1)
                    .to_broadcast([128, JPP, NCLS]),
                    t_i20[:].rearrange("p (one c) -> p one c", one=1)
                    .to_broadcast([128, JPP, NCLS]),
                    ALU.is_equal)
                t20v = t_t20[:].rearrange("p (j c) -> p j c", c=NCLS)
                for a in range(NA):
                    nc.vector.tensor_tensor(t20v, qclsv[:, :, a, :], ohv,
                                            ALU.mult)
                    nc.vector.tensor_reduce(
                        r5(t_qcl)[:, :, a], t20v, axis=AX.X, op=ALU.add)
                # S2 per anchor from squares, then fuse and winner-select
                t_sq = t_scr  # reuse
                for a in range(NA):
                    nc.vector.tensor_tensor(t20v, qclsv[:, :, a, :],
                                            qclsv[:, :, a, :], ALU.mult)
                    nc.vector.tensor_reduce(
                        r5(t_sq)[:, :, a], t20v, axis=AX.X, op=ALU.add)
                # cls_t = S2 - 2*qcls  (the +1 handled via sum(win))
                nc.vector.scalar_tensor_tensor(
                    out=t_scr2[:], in0=t_qcl[:], scalar=-2.0, in1=t_sq[:],
                    op0=ALU.mult, op1=ALU.add)
                nc.vector.tensor_mul(t_scr2[:], t_scr2[:], t_ohA[:])
                nc.vector.tensor_reduce(s_cls[:, jsl], r5(t_scr2), axis=AX.X,
                                        op=ALU.add)

            # ---- single-shot last-writer-wins dedup over all objects ----
            t_sT = wk.tile([J2, 128], f32)
            t_eqp = wk.tile([J2, 4 * O * O], f32)
            t_deadT = wk.tile([J2, 128], f32)
            t_tp1 = psA.tile([J2, 128], f32, space="PSUM", tag="ded", bufs=1)
            nc.tensor.transpose(out=t_tp1[:], in_=t_sid[:], identity=t_id[:])
            nc.scalar.activation(t_sT[:], t_tp1[:], ACT.Copy)
            sTa = t_sT[:].rearrange("p (i o one) -> p i o one", i=4, one=1) \
                         .to_broadcast([J2, 4, O, O])
            sTb = t_sT[:].rearrange("p (i one o2) -> p i one o2", i=4, one=1) \
                         .to_broadcast([J2, 4, O, O])
            eqv = t_eqp[:].rearrange("p (i o o2) -> p i o o2", i=4, o2=O)
            nc.vector.tensor_tensor(eqv, sTa, sTb, ALU.is_equal)
            triv = t_tri[0:J2, :].rearrange(
                "p (one o o2) -> p one o o2", one=1, o2=O) \
                .to_broadcast([J2, 4, O, O])
            nc.vector.tensor_tensor(eqv, eqv, triv, ALU.mult)
            nc.vector.tensor_reduce(
                t_deadT[:].rearrange("p (i o) -> p i o", o=O),
                eqv, axis=AX.X, op=ALU.max)
            t_tp2 = psA.tile([128, J2], f32, space="PSUM", tag="ded2", bufs=1)
            nc.tensor.transpose(
                out=t_tp2[:], in_=t_deadT[:], identity=t_id[0:J2, 0:J2])
            nc.scalar.activation(t_win[:], t_tp2[:], ACT.Copy)
            nc.vector.tensor_scalar(
                t_win[:], t_win[:], -1.0, 1.0, ALU.mult, ALU.add)

            # ---- win-masked partial sums into staging ----
            t_red1 = wk.tile([128, 1], f32)

            def accw(col, stash):
                nc.vector.tensor_mul(stash[:], stash[:], t_win[:])
                nc.vector.reduce_sum(t_red1[:], stash[:], axis=AX.X)
                nc.vector.tensor_copy(t_stage[:, col:col + 1], t_red1[:])
            accw(0, s_csse)   # coord SSE (unweighted by 5)
            accw(1, s_c1)     # (1-q4)^2 at slots
            accw(2, s_q4)     # q4^2 at slots
            accw(3, s_cls)    # S2 - 2*qcls at slots
            nc.vector.reduce_sum(t_red1[:], t_win[:], axis=AX.X)
            nc.vector.tensor_copy(t_stage[:, 4:5], t_red1[:])

            nc.sync.dma_start(out[:], t_stage[:])

    nc.compile()
    return nc


def _get_built():
    if "nc" not in _CACHE:
        _CACHE["nc"] = _build()
        _CACHE["consts"] = _make_consts()
    return _CACHE["nc"], _CACHE["consts"]


def _reduce_partials(P):
    """P: [ncores, 128, 16] fp32 partials -> the 4 scalar losses."""
    S = P.astype(np.float64).sum(axis=(0, 1))
    coord, confobj, confsub, clsq, wsum = S[0], S[1], S[2], S[3], S[4]
    dense = S[5:10].sum()
    obj_loss = 5.0 * coord + confobj
    no_obj_loss = 0.5 * (dense - confsub)
    conf_loss = clsq + wsum
    loss = obj_loss + no_obj_loss + conf_loss
    return (np.float32(loss), np.float32(obj_loss),
            np.float32(no_obj_loss), np.float32(conf_loss))


def kernel(detection_result, gt_boxes, gt_class):
    from concourse.bass_utils import run_bass_kernel_spmd

    nc, consts = _get_built()
    det = np.ascontiguousarray(
        np.asarray(detection_result, dtype=np.float32)).reshape(B, NCH, CELLS)
    # cell-major, channel-padded copy for the 512B-row object gathers
    detT = np.zeros((B, CELLS, CHP), dtype=np.float32)
    detT[:, :, :NCH] = det.transpose(0, 2, 1)
    detT = detT.reshape(B * CELLS, CHP)
    confs = np.ascontiguousarray(det[:, 4::CH, :])  # [B, 5, 169]
    gtb = np.ascontiguousarray(np.asarray(gt_boxes, dtype=np.float32))
    clsf = np.asarray(gt_class).astype(np.float32)

    in_maps = []
    for c in range(NCORES):
        sl = slice(c * BLOC, (c + 1) * BLOC)
        m = {"detT": detT[c * BLOC * CELLS:(c + 1) * BLOC * CELLS],
             "conf": confs[sl], "gtb": gtb[sl], "clsf": clsf[sl]}
        m.update(consts)
        in_maps.append(m)

    res = run_bass_kernel_spmd(nc, in_maps, core_ids=list(range(NCORES)))
    _CACHE["last_res"] = res
    P = np.stack([res.results[c]["out"] for c in range(NCORES)])
    return _reduce_partials(P)
